# revision 1
# baseline (speedup 1.0000x reference)
"""AngleEmbedding kernel for 8 TRN2 NeuronCores.

The reference applies, per qubit q, the overwrite-semantics "rotation"
    new[i0] = 1j*sin(th/2)*state[i1];  new[i1] = cos(th/2)*state[i1]
(i1 = index with bit q set). Both outputs depend only on the bit=1
amplitudes. The initial state |0...0> has zero amplitude at every index
with any bit set, so the state is identically zero after the first
rotation and stays zero: the exact output is zeros((8, 2^20), complex64)
for every input x.

The kernel therefore reduces to materializing the 64 MiB zero output at
HBM write bandwidth. Sharding (per the state-vector-parallel hint): the
2^20 state axis is split across the 8 cores; each core owns 2^17 states
per batch row = 8 MiB of f32 (re,im) pairs and writes them with large
HWDGE DMAs from a memset SBUF tile.
"""

import numpy as np

N_CORES = 8
BATCH = 8
N_QUBITS = 20
STATES = 1 << N_QUBITS                      # 1048576
SHARD_STATES = STATES // N_CORES            # 131072 states per core
SHARD_F32 = BATCH * SHARD_STATES * 2        # 2097152 f32 per core (8 MiB)
OUT_P = 128
OUT_F = SHARD_F32 // OUT_P                  # 16384
CHUNK_F = 2048                              # [128, 2048] f32 = 1 MiB zero tile

_CACHE = {}


def _build_nc():
    import concourse.bass as bass
    import concourse.mybir as mybir

    nc = bass.Bass()
    x = nc.declare_dram_parameter(
        "x", [BATCH, N_QUBITS], mybir.dt.float32, isOutput=False
    )
    out = nc.declare_dram_parameter(
        "out", [OUT_P, OUT_F], mybir.dt.float32, isOutput=True
    )
    n_chunks = OUT_F // CHUNK_F

    with (
        nc.sbuf_tensor([OUT_P, CHUNK_F], mybir.dt.float32) as ztile,
        nc.sbuf_tensor([BATCH, N_QUBITS], mybir.dt.float32) as xtile,
        nc.semaphore() as zsem,
        nc.semaphore() as dsem,
        nc.semaphore() as xsem,
        nc.Block() as block,
    ):
        @block.gpsimd
        def _(gpsimd):
            # Consume the angle input (the output is independent of it).
            gpsimd.dma_start(out=xtile[:], in_=x[:]).then_inc(xsem, 16)
            gpsimd.wait_ge(xsem, 16)

        @block.vector
        def _(vector):
            vector.memset(ztile[:], 0.0).then_inc(zsem, 1)

        @block.sync
        def _(sync):
            sync.wait_ge(zsem, 1)
            for k in range(n_chunks):
                sync.dma_start(
                    out=out[:, k * CHUNK_F:(k + 1) * CHUNK_F], in_=ztile[:]
                ).then_inc(dsem, 16)
            sync.wait_ge(dsem, 16 * n_chunks)

    return nc


def _run(x, trace=False):
    from concourse.bass_utils import run_bass_kernel_spmd

    if "nc" not in _CACHE:
        _CACHE["nc"] = _build_nc()
    nc = _CACHE["nc"]

    xf = np.ascontiguousarray(np.asarray(x, dtype=np.float32))
    assert xf.shape == (BATCH, N_QUBITS)
    in_maps = [{"x": xf} for _ in range(N_CORES)]
    res = run_bass_kernel_spmd(
        nc, in_maps, core_ids=list(range(N_CORES)), trace=trace
    )
    # Core i holds states [i*SHARD_STATES, (i+1)*SHARD_STATES) for each
    # batch row, as interleaved (re, im) f32 pairs.
    parts = [
        res.results[i]["out"].reshape(BATCH, SHARD_STATES * 2)
        for i in range(N_CORES)
    ]
    full = np.ascontiguousarray(np.concatenate(parts, axis=1))
    return full.view(np.complex64), res


def kernel(x):
    out, _ = _run(x, trace=False)
    return out


# revision 2
# speedup vs baseline: 1.0112x; 1.0112x over previous
"""AngleEmbedding kernel for 8 TRN2 NeuronCores.

The reference applies, per qubit q, the overwrite-semantics "rotation"
    new[i0] = 1j*sin(th/2)*state[i1];  new[i1] = cos(th/2)*state[i1]
(i1 = index with bit q set). Both outputs depend only on the bit=1
amplitudes. The initial state |0...0> has zero amplitude at every index
with any bit set, so the state is identically zero after the first
rotation and stays zero: the exact output is zeros((8, 2^20), complex64)
for every input x.

The kernel therefore reduces to materializing the 64 MiB zero output at
HBM write bandwidth. Sharding (per the state-vector-parallel hint): the
2^20 state axis is split across the 8 cores; each core owns 2^17 states
per batch row = 8 MiB of f32 (re,im) pairs.

Per-core schedule: a small [128, 1024] f32 SBUF tile is memset to zero
(emitted before the Block so it issues right after NEFF init), then the
two HWDGE engines (sync, scalar) each stream half the 8 MiB to DRAM with
step-0 repeat source APs reading that one tile.
"""

import numpy as np

N_CORES = 8
BATCH = 8
N_QUBITS = 20
STATES = 1 << N_QUBITS                      # 1048576
SHARD_STATES = STATES // N_CORES            # 131072 states per core
SHARD_F32 = BATCH * SHARD_STATES * 2        # 2097152 f32 per core (8 MiB)
OUT_P = 128
OUT_F = SHARD_F32 // OUT_P                  # 16384
TILE_F = 1024                               # [128, 1024] f32 = 512 KiB zero tile
DMAS_PER_ENGINE = 2                         # chunks per HWDGE engine

_CACHE = {}


def _build_nc():
    import concourse.bass as bass
    import concourse.mybir as mybir

    nc = bass.Bass()
    x = nc.declare_dram_parameter(
        "x", [BATCH, N_QUBITS], mybir.dt.float32, isOutput=False
    )
    out = nc.declare_dram_parameter(
        "out", [OUT_P, OUT_F], mybir.dt.float32, isOutput=True
    )

    n_chunks = 2 * DMAS_PER_ENGINE
    chunk_f = OUT_F // n_chunks
    rep = chunk_f // TILE_F

    with (
        nc.sbuf_tensor([OUT_P, TILE_F], mybir.dt.float32) as ztile,
        nc.sbuf_tensor([BATCH, N_QUBITS], mybir.dt.float32) as xtile,
        nc.semaphore() as zsem,
        nc.semaphore() as dsem,
        nc.semaphore() as xsem,
    ):
        t = ztile[:]
        # Source AP reading the zero tile `rep` times: partition dim first,
        # then a step-0 repeat dim over the TILE_F-element run.
        zrep = bass.AP(t.tensor, t.offset, [list(t.ap[0]), [0, rep], list(t.ap[1])])

        # Emitted before the Block: lands right after NEFF init on DVE.
        nc.vector.memset(t, 0.0).then_inc(zsem, 1)

        def issue(engine, chunk_ids):
            engine.wait_ge(zsem, 1)
            for k in chunk_ids:
                dst = out[:, k * chunk_f:(k + 1) * chunk_f]
                engine.dma_start(
                    out=dst.rearrange("p (r f) -> p r f", r=rep), in_=zrep
                ).then_inc(dsem, 16)

        with nc.Block() as block:
            @block.gpsimd
            def _(gpsimd):
                # Consume the angle input (the output is independent of it).
                gpsimd.dma_start(out=xtile[:], in_=x[:]).then_inc(xsem, 16)
                gpsimd.wait_ge(xsem, 16)

            @block.scalar
            def _(scalar):
                issue(scalar, range(DMAS_PER_ENGINE, n_chunks))

            @block.sync
            def _(sync):
                issue(sync, range(DMAS_PER_ENGINE))
                sync.wait_ge(dsem, 16 * n_chunks)

    return nc


def _run(x, trace=False):
    from concourse.bass_utils import run_bass_kernel_spmd

    if "nc" not in _CACHE:
        _CACHE["nc"] = _build_nc()
    nc = _CACHE["nc"]

    xf = np.ascontiguousarray(np.asarray(x, dtype=np.float32))
    assert xf.shape == (BATCH, N_QUBITS)
    in_maps = [{"x": xf} for _ in range(N_CORES)]
    res = run_bass_kernel_spmd(
        nc, in_maps, core_ids=list(range(N_CORES)), trace=trace
    )
    # Core i holds states [i*SHARD_STATES, (i+1)*SHARD_STATES) for each
    # batch row, as interleaved (re, im) f32 pairs.
    parts = [
        res.results[i]["out"].reshape(BATCH, SHARD_STATES * 2)
        for i in range(N_CORES)
    ]
    full = np.ascontiguousarray(np.concatenate(parts, axis=1))
    return full.view(np.complex64), res


def kernel(x):
    out, _ = _run(x, trace=False)
    return out
